# revision 11
# baseline (speedup 1.0000x reference)
"""Trainium2 Bass kernel for attention-pooling:
    score  = tanh(X @ W)            [B,T,H]
    logits = score @ c              [B,T]
    attn   = softmax(logits, ax=1)  [B,T]
    ctx    = attn-weighted sum of X over T -> [B,H]
Returns (ctx, attn). Data-parallel over batch across 8 NeuronCores.

Per-core pipeline (4 samples, T=2048, H=1024):
  P0   gpsimd cast-DMA x f32 -> DRAM scratch bf16, tiled [hb][t][128]
  XT   one huge xbar DMA-transpose per (sample, h-block): [2048,128]->[128,2048]
  A    S^T = W^T X^T in bf16 (W stationary), tanh fused on ScalarE
  L    logits^T via N=1 matmuls -> [128t, 1] slices (softmax-friendly layout)
  SM   exp (no max-sub: |logits| <= sum|c| ~ 40), cross-partition sum via
       ones-matmul, reciprocal broadcast, attn = probs * recip
  P    ctx = attn^T X with attn[128,1] stationary tiles, X natural from scratch
"""

import numpy as np

B, T, H = 32, 2048, 1024
NCORES = 8
BC = B // NCORES            # samples per core
CHUNK = 512                 # t-rows per phase-A chunk
CH_PER_B = T // CHUNK       # 4 chunks per sample
HB = H // 128               # 8 h blocks
JJ = CHUNK // 128           # 4 128-t slices per chunk
SPS = T // 128              # 16 slices per sample

_cache = {}


def build():
    import concourse.bass as bass
    import concourse.tile as tile
    from concourse import bacc, mybir
    from contextlib import ExitStack

    f32 = mybir.dt.float32
    bf16 = mybir.dt.bfloat16
    AF = mybir.ActivationFunctionType
    AX = mybir.AxisListType

    nc = bacc.Bacc("TRN2", target_bir_lowering=False, debug=False)

    x = nc.declare_dram_parameter("x", [BC, T, H], f32, isOutput=False)
    w = nc.declare_dram_parameter("w", [H, H], f32, isOutput=False)
    c = nc.declare_dram_parameter("c", [H, 1], f32, isOutput=False)
    out_ctx = nc.declare_dram_parameter("out_ctx", [BC, H], f32, isOutput=True)
    out_attn = nc.declare_dram_parameter("out_attn", [BC, T], f32, isOutput=True)

    xf = x[:].rearrange("b t h -> (b t) h")  # [8192, 1024]

    with tile.TileContext(nc) as tc, ExitStack() as ctx:
        ep = ctx.enter_context
        const_pool = ep(tc.tile_pool(name="const", bufs=1))
        xt_pool = ep(tc.tile_pool(name="xt", bufs=16))
        st_pool = ep(tc.tile_pool(name="st", bufs=16))
        nat_pool = ep(tc.tile_pool(name="nat", bufs=6))
        sm_pool = ep(tc.tile_pool(name="sm", bufs=2))
        dram_pool = ep(tc.tile_pool(name="dram", bufs=4, space="DRAM"))
        mm_ps = ep(tc.tile_pool(name="mm_ps", bufs=3, space="PSUM"))
        lg_ps = ep(tc.tile_pool(name="lg_ps", bufs=1, space="PSUM"))
        ctx_ps = ep(tc.tile_pool(name="ctx_ps", bufs=2, space="PSUM"))
        tiny_ps = ep(tc.tile_pool(name="tiny_ps", bufs=2, space="PSUM"))

        # ---- constants / weights ----
        w_sb = const_pool.tile([128, HB, H], bf16, tag="w")
        nc.gpsimd.dma_start(
            out=w_sb[:], in_=w[:].rearrange("(hb p) h -> p hb h", p=128)
        )
        c_sb = const_pool.tile([128, HB], bf16, tag="c")
        nc.gpsimd.dma_start(
            out=c_sb[:], in_=c[:].rearrange("(a p) k -> p (a k)", p=128)
        )
        ones_col = const_pool.tile([128, 1], f32, tag="ones_col")
        nc.any.memset(ones_col[:], 1.0)
        ones_row = const_pool.tile([1, 128], f32, tag="ones_row")
        nc.any.memset(ones_row[:], 1.0)
        probs = const_pool.tile([128, BC * SPS], f32, tag="probs")

        xbf_by_b = [None] * BC
        cast_insts_by_b = [None] * BC

        def sample_tail(b):
            """softmax + pooling for sample b (all its logits are in probs)."""
            pcols = probs[:, b * SPS : (b + 1) * SPS]
            partial = sm_pool.tile([128, 1], f32, tag="partial")
            nc.vector.reduce_sum(partial[:], pcols, axis=AX.X)
            tps = tiny_ps.tile([1, 1], f32, tag="tiny")
            nc.tensor.matmul(tps[:], ones_col[:], partial[:], start=True, stop=True)
            tsb = sm_pool.tile([1, 1], f32, tag="tsb")
            nc.vector.tensor_copy(tsb[:], tps[:])
            bps = tiny_ps.tile([128, 1], f32, tag="tiny")
            nc.tensor.matmul(bps[:], ones_row[:], tsb[:], start=True, stop=True)
            rsb = sm_pool.tile([128, 1], f32, tag="rsb")
            nc.vector.reciprocal(rsb[:], bps[:])
            attn_f = sm_pool.tile([128, SPS], f32, tag="attn_f")
            nc.vector.tensor_scalar_mul(attn_f[:], pcols, rsb[:])
            attn_b = sm_pool.tile([128, SPS], bf16, tag="attn_b")
            nc.vector.tensor_copy(attn_b[:], attn_f[:])
            nc.scalar.dma_start(
                out=out_attn[b].rearrange("(s p) -> p s", p=128), in_=attn_f[:]
            )
            # pooling: ctx[1, H] = sum_s attn[s-slice]^T @ X[s-slice, :]
            xbf = xbf_by_b[b]
            cps = [
                ctx_ps.tile([1, 512], f32, tag="ctx", name=f"cps{hh}")
                for hh in range(2)
            ]
            for s in range(SPS):
                nat = nat_pool.tile([128, HB, 128], bf16, tag="nat")
                ni = nc.gpsimd.dma_start(
                    out=nat[:],
                    in_=xbf[:, s * 128 : (s + 1) * 128, :].rearrange(
                        "hb t h -> t hb h"
                    ),
                )
                for ci in cast_insts_by_b[b]:
                    tile.add_dep_helper(ni.ins, ci.ins, reason="xbf RAW nat")
                for hh in range(2):
                    nc.tensor.matmul(
                        cps[hh][:],
                        attn_b[:, s : s + 1],
                        nat[:, hh * 4 : (hh + 1) * 4, :],
                        start=(s == 0),
                        stop=(s == SPS - 1),
                    )
            ctxs = sm_pool.tile([1, H], f32, tag="ctxs")
            for hh in range(2):
                nc.vector.tensor_copy(ctxs[:, hh * 512 : (hh + 1) * 512], cps[hh][:])
            nc.scalar.dma_start(out=out_ctx[b : b + 1, :], in_=ctxs[:])

        def main_mms(b, ch, xts):
            """main matmuls + tanh for chunk ch of sample b; returns st tiles."""
            sts = []
            for hob in range(HB):
                ps = mm_ps.tile([128, CHUNK], f32, tag="mm")
                for hib in range(HB):
                    nc.tensor.matmul(
                        ps[:],
                        w_sb[:, hib, hob * 128 : (hob + 1) * 128],
                        xts[hib][:, ch * CHUNK : (ch + 1) * CHUNK],
                        start=(hib == 0),
                        stop=(hib == HB - 1),
                    )
                st = st_pool.tile([128, CHUNK], bf16, tag="st")
                nc.scalar.activation(st[:], ps[:], AF.Tanh)
                sts.append(st)
            return sts

        def logits_part(b, ch, sts):
            """logits^T matmuls + exp for chunk ch of sample b."""
            lg = lg_ps.tile([128, JJ], f32, tag="lg")
            for j in range(JJ):
                for hob in range(HB):
                    nc.tensor.matmul(
                        lg[:, j : j + 1],
                        sts[hob][:, j * 128 : (j + 1) * 128],
                        c_sb[:, hob : hob + 1],
                        start=(hob == 0),
                        stop=(hob == HB - 1),
                    )
            gs = b * SPS + ch * JJ  # global slice index
            nc.scalar.activation(probs[:, gs : gs + JJ], lg[:], AF.Exp)

        def emit_transposes(b):
            # XT: one xbar transpose per h-block, all on the SP ring — two
            # concurrent xbar streams (sync + scalar) corrupt data.
            xts = []
            for hb in range(HB):
                xt = xt_pool.tile([128, T], bf16, tag="xt", name=f"xt{b}_{hb}")
                ti = nc.sync.dma_start(out=xt[:], in_=xbf_by_b[b][hb], transpose=True)
                tile.add_dep_helper(
                    ti.ins, cast_insts_by_b[b][hb].ins, reason="xbf RAW"
                )
                xts.append(xt)
            return xts

        # P0 upfront: cast all samples' x f32 -> bf16 DRAM scratch (gpsimd)
        for b in range(BC):
            xbf = dram_pool.tile([HB, T, 128], bf16, tag="xbf", name=f"xbf{b}")
            xbf_by_b[b] = xbf
            cast_insts = []
            for hb in range(HB):
                ci = nc.gpsimd.dma_start(
                    out=xbf[hb],
                    in_=xf[b * T : (b + 1) * T, hb * 128 : (hb + 1) * 128],
                )
                cast_insts.append(ci)
            cast_insts_by_b[b] = cast_insts

        # Software-pipelined emission: transposes one sample ahead; logits one
        # chunk behind main MMs; sample tails two chunks behind.
        xts_by_b = {0: emit_transposes(0)}
        pend_lg = None   # (b, ch, sts) awaiting logits emission
        pend_tail = None  # sample awaiting softmax+pooling emission
        for b in range(BC):
            if b + 1 < BC:
                xts_by_b[b + 1] = emit_transposes(b + 1)
            for ch in range(CH_PER_B):
                sts = main_mms(b, ch, xts_by_b[b])
                if pend_lg is not None:
                    logits_part(*pend_lg)
                pend_lg = (b, ch, sts)
                if ch == 1 and pend_tail is not None:
                    sample_tail(pend_tail)
                    pend_tail = None
            pend_tail = b
        logits_part(*pend_lg)
        sample_tail(pend_tail)

    nc.compile()
    return nc


def _get_nc():
    if "nc" not in _cache:
        _cache["nc"] = build()
    return _cache["nc"]


def kernel(gru_output, attention_weights, context_vector):
    from concourse.bass_utils import run_bass_kernel_spmd

    nc = _get_nc()
    in_maps = []
    for i in range(NCORES):
        in_maps.append(
            {
                "x": np.ascontiguousarray(
                    gru_output[i * BC : (i + 1) * BC], dtype=np.float32
                ),
                "w": np.ascontiguousarray(attention_weights, dtype=np.float32),
                "c": np.ascontiguousarray(context_vector, dtype=np.float32),
            }
        )
    res = run_bass_kernel_spmd(nc, in_maps, list(range(NCORES))).results
    context = np.concatenate([res[i]["out_ctx"] for i in range(NCORES)], axis=0)
    attn = np.concatenate([res[i]["out_attn"] for i in range(NCORES)], axis=0)
    return context, attn


# revision 13
# speedup vs baseline: 1.0574x; 1.0574x over previous
"""Trainium2 Bass kernel for attention-pooling:
    score  = tanh(X @ W)            [B,T,H]
    logits = score @ c              [B,T]
    attn   = softmax(logits, ax=1)  [B,T]
    ctx    = attn-weighted sum of X over T -> [B,H]
Returns (ctx, attn). Data-parallel over batch across 8 NeuronCores.

Per-core pipeline (4 samples, T=2048, H=1024):
  P0   gpsimd cast-DMA x f32 -> DRAM scratch bf16, tiled [hb][t][128]
  XT   one huge xbar DMA-transpose per (sample, h-block): [2048,128]->[128,2048]
  A    S^T = W^T X^T in bf16 (W stationary), tanh fused on ScalarE
  L    logits^T via N=1 matmuls -> [128t, 1] slices (softmax-friendly layout)
  SM   exp (no max-sub: |logits| <= sum|c| ~ 40), cross-partition sum via
       ones-matmul, reciprocal broadcast, attn = probs * recip
  P    ctx = attn^T X with attn[128,1] stationary tiles, X natural from scratch
"""

import numpy as np

B, T, H = 32, 2048, 1024
NCORES = 8
BC = B // NCORES            # samples per core
CHUNK = 512                 # t-rows per phase-A chunk
CH_PER_B = T // CHUNK       # 4 chunks per sample
HB = H // 128               # 8 h blocks
JJ = CHUNK // 128           # 4 128-t slices per chunk
SPS = T // 128              # 16 slices per sample

_cache = {}


def build():
    import concourse.bass as bass
    import concourse.tile as tile
    from concourse import bacc, mybir
    from contextlib import ExitStack

    f32 = mybir.dt.float32
    bf16 = mybir.dt.bfloat16
    AF = mybir.ActivationFunctionType
    AX = mybir.AxisListType

    nc = bacc.Bacc("TRN2", target_bir_lowering=False, debug=False)

    x = nc.declare_dram_parameter("x", [BC, T, H], f32, isOutput=False)
    w = nc.declare_dram_parameter("w", [H, H], f32, isOutput=False)
    c = nc.declare_dram_parameter("c", [H, 1], f32, isOutput=False)
    out_ctx = nc.declare_dram_parameter("out_ctx", [BC, H], f32, isOutput=True)
    out_attn = nc.declare_dram_parameter("out_attn", [BC, T], f32, isOutput=True)

    xf = x[:].rearrange("b t h -> (b t) h")  # [8192, 1024]

    with tile.TileContext(nc) as tc, ExitStack() as ctx:
        ep = ctx.enter_context
        const_pool = ep(tc.tile_pool(name="const", bufs=1))
        xt_pool = ep(tc.tile_pool(name="xt", bufs=16))
        st_pool = ep(tc.tile_pool(name="st", bufs=16))
        nat_pool = ep(tc.tile_pool(name="nat", bufs=6))
        sm_pool = ep(tc.tile_pool(name="sm", bufs=2))
        dram_pool = ep(tc.tile_pool(name="dram", bufs=4, space="DRAM"))
        mm_ps = ep(tc.tile_pool(name="mm_ps", bufs=3, space="PSUM"))
        lg_ps = ep(tc.tile_pool(name="lg_ps", bufs=1, space="PSUM"))
        ctx_ps = ep(tc.tile_pool(name="ctx_ps", bufs=2, space="PSUM"))
        tiny_ps = ep(tc.tile_pool(name="tiny_ps", bufs=2, space="PSUM"))

        # ---- constants / weights ----
        w_sb = const_pool.tile([128, HB, H], bf16, tag="w")
        nc.gpsimd.dma_start(
            out=w_sb[:], in_=w[:].rearrange("(hb p) h -> p hb h", p=128)
        )
        c_sb = const_pool.tile([128, HB], bf16, tag="c")
        nc.gpsimd.dma_start(
            out=c_sb[:], in_=c[:].rearrange("(a p) k -> p (a k)", p=128)
        )
        ones_col = const_pool.tile([128, 1], f32, tag="ones_col")
        nc.any.memset(ones_col[:], 1.0)
        ones_row = const_pool.tile([1, 128], f32, tag="ones_row")
        nc.any.memset(ones_row[:], 1.0)
        probs = const_pool.tile([128, BC * SPS], f32, tag="probs")

        xbf_by_b = [None] * BC
        cast_insts_by_b = [None] * BC

        def sample_tail(b):
            """softmax + pooling for sample b (all its logits are in probs)."""
            pcols = probs[:, b * SPS : (b + 1) * SPS]
            partial = sm_pool.tile([128, 1], f32, tag="partial")
            nc.vector.reduce_sum(partial[:], pcols, axis=AX.X)
            tps = tiny_ps.tile([1, 1], f32, tag="tiny")
            nc.tensor.matmul(tps[:], ones_col[:], partial[:], start=True, stop=True)
            tsb = sm_pool.tile([1, 1], f32, tag="tsb")
            nc.vector.tensor_copy(tsb[:], tps[:])
            bps = tiny_ps.tile([128, 1], f32, tag="tiny")
            nc.tensor.matmul(bps[:], ones_row[:], tsb[:], start=True, stop=True)
            rsb = sm_pool.tile([128, 1], f32, tag="rsb")
            nc.vector.reciprocal(rsb[:], bps[:])
            attn_f = sm_pool.tile([128, SPS], f32, tag="attn_f")
            nc.vector.tensor_scalar_mul(attn_f[:], pcols, rsb[:])
            attn_b = sm_pool.tile([128, SPS], bf16, tag="attn_b")
            nc.vector.tensor_copy(attn_b[:], attn_f[:])
            nc.scalar.dma_start(
                out=out_attn[b].rearrange("(s p) -> p s", p=128), in_=attn_f[:]
            )
            # pooling: ctx[1, H] = sum_s attn[s-slice]^T @ X[s-slice, :]
            xbf = xbf_by_b[b]
            cps = [
                ctx_ps.tile([1, 512], f32, tag="ctx", name=f"cps{hh}")
                for hh in range(2)
            ]
            for s in range(SPS):
                nat = nat_pool.tile([128, H], bf16, tag="nat")
                ni = nc.gpsimd.dma_start(
                    out=nat[:], in_=xbf[s * 128 : (s + 1) * 128, :]
                )
                tile.add_dep_helper(
                    ni.ins, cast_insts_by_b[b][s // JJ].ins, reason="xnat RAW nat"
                )
                for hh in range(2):
                    nc.tensor.matmul(
                        cps[hh][:],
                        attn_b[:, s : s + 1],
                        nat[:, hh * 512 : (hh + 1) * 512],
                        start=(s == 0),
                        stop=(s == SPS - 1),
                    )
            ctxs = sm_pool.tile([1, H], f32, tag="ctxs")
            for hh in range(2):
                nc.vector.tensor_copy(ctxs[:, hh * 512 : (hh + 1) * 512], cps[hh][:])
            nc.scalar.dma_start(out=out_ctx[b : b + 1, :], in_=ctxs[:])

        def main_mms(b, ch, xts):
            """main matmuls + tanh for chunk ch of sample b; returns st tiles."""
            sts = []
            for hob in range(HB):
                ps = mm_ps.tile([128, CHUNK], f32, tag="mm")
                for hib in range(HB):
                    nc.tensor.matmul(
                        ps[:],
                        w_sb[:, hib, hob * 128 : (hob + 1) * 128],
                        xts[hib][:, ch * CHUNK : (ch + 1) * CHUNK],
                        start=(hib == 0),
                        stop=(hib == HB - 1),
                    )
                st = st_pool.tile([128, CHUNK], bf16, tag="st")
                nc.scalar.activation(st[:], ps[:], AF.Tanh)
                sts.append(st)
            return sts

        def logits_part(b, ch, sts):
            """logits^T matmuls + exp for chunk ch of sample b."""
            lg = lg_ps.tile([128, JJ], f32, tag="lg")
            for j in range(JJ):
                for hob in range(HB):
                    nc.tensor.matmul(
                        lg[:, j : j + 1],
                        sts[hob][:, j * 128 : (j + 1) * 128],
                        c_sb[:, hob : hob + 1],
                        start=(hob == 0),
                        stop=(hob == HB - 1),
                    )
            gs = b * SPS + ch * JJ  # global slice index
            nc.scalar.activation(probs[:, gs : gs + JJ], lg[:], AF.Exp)

        def emit_casts(b):
            """P0: cast sample b's x f32 -> bf16 natural-layout DRAM scratch.
            Contiguous reads/writes — one cast per 512-row chunk."""
            xbf = dram_pool.tile([T, H], bf16, tag="xbf", name=f"xbf{b}")
            xbf_by_b[b] = xbf
            cast_insts = []
            for ch in range(CH_PER_B):
                ci = nc.gpsimd.dma_start(
                    out=xbf[ch * CHUNK : (ch + 1) * CHUNK, :],
                    in_=xf[b * T + ch * CHUNK : b * T + (ch + 1) * CHUNK, :],
                )
                cast_insts.append(ci)
            cast_insts_by_b[b] = cast_insts

        def emit_transposes(b, half, xts=None):
            # XT: xbar transposes, all on the SP ring — two concurrent xbar
            # streams (sync + scalar) corrupt data. The strided-row source
            # (256B rows, 2KB stride) is the xbar's supported mid-dim case.
            if xts is None:
                xts = [
                    xt_pool.tile([128, T], bf16, tag="xt", name=f"xt{b}_{hb}")
                    for hb in range(HB)
                ]
            t0, t1 = half * (T // 2), (half + 1) * (T // 2)
            for hb in range(HB):
                ti = nc.sync.dma_start(
                    out=xts[hb][:, t0:t1],
                    in_=xbf_by_b[b][t0:t1, hb * 128 : (hb + 1) * 128],
                    transpose=True,
                )
                for ci in cast_insts_by_b[b][half * 2 : half * 2 + 2]:
                    tile.add_dep_helper(ti.ins, ci.ins, reason="xnat RAW")
            return xts

        # Software-pipelined emission: casts one sample ahead; transposes a
        # half-sample ahead; logits one chunk behind main MMs; sample tails
        # lag into the next sample's second chunk.
        emit_casts(0)
        xts_by_b = {0: emit_transposes(0, 0)}
        emit_transposes(0, 1, xts_by_b[0])
        pend_lg = None   # (b, ch, sts) awaiting logits emission
        pend_tail = None  # sample awaiting softmax+pooling emission
        for b in range(BC):
            if b + 1 < BC:
                emit_casts(b + 1)
                xts_by_b[b + 1] = emit_transposes(b + 1, 0)
            for ch in range(CH_PER_B):
                if ch == 2 and b + 1 < BC:
                    emit_transposes(b + 1, 1, xts_by_b[b + 1])
                sts = main_mms(b, ch, xts_by_b[b])
                if pend_lg is not None:
                    logits_part(*pend_lg)
                pend_lg = (b, ch, sts)
                if ch == 1 and pend_tail is not None:
                    sample_tail(pend_tail)
                    pend_tail = None
            xts_by_b.pop(b - 1, None)
            pend_tail = b
        logits_part(*pend_lg)
        sample_tail(pend_tail)

    nc.compile()
    return nc


def _get_nc():
    if "nc" not in _cache:
        _cache["nc"] = build()
    return _cache["nc"]


def kernel(gru_output, attention_weights, context_vector):
    from concourse.bass_utils import run_bass_kernel_spmd

    nc = _get_nc()
    in_maps = []
    for i in range(NCORES):
        in_maps.append(
            {
                "x": np.ascontiguousarray(
                    gru_output[i * BC : (i + 1) * BC], dtype=np.float32
                ),
                "w": np.ascontiguousarray(attention_weights, dtype=np.float32),
                "c": np.ascontiguousarray(context_vector, dtype=np.float32),
            }
        )
    res = run_bass_kernel_spmd(nc, in_maps, list(range(NCORES))).results
    context = np.concatenate([res[i]["out_ctx"] for i in range(NCORES)], axis=0)
    attn = np.concatenate([res[i]["out_attn"] for i in range(NCORES)], axis=0)
    return context, attn


# revision 20
# speedup vs baseline: 1.2394x; 1.1721x over previous
"""Trainium2 Bass kernel for attention-pooling:
    score  = tanh(X @ W)            [B,T,H]
    logits = score @ c              [B,T]
    attn   = softmax(logits, ax=1)  [B,T]
    ctx    = attn-weighted sum of X over T -> [B,H]
Returns (ctx, attn). Data-parallel over batch across 8 NeuronCores.

Per-core pipeline (4 samples, T=2048, H=1024):
  P0   gpsimd cast-DMA x f32 -> DRAM scratch bf16, tiled [hb][t][128]
  XT   one huge xbar DMA-transpose per (sample, h-block): [2048,128]->[128,2048]
  A    S^T = W^T X^T in bf16 (W stationary), tanh fused on ScalarE
  L    logits^T via N=1 matmuls -> [128t, 1] slices (softmax-friendly layout)
  SM   exp (no max-sub: |logits| <= sum|c| ~ 40), cross-partition sum via
       ones-matmul, reciprocal broadcast, attn = probs * recip
  P    ctx = attn^T X with attn[128,1] stationary tiles, X natural from scratch
"""

import numpy as np

B, T, H = 32, 2048, 1024
NCORES = 8
BC = B // NCORES            # samples per core
CHUNK = 512                 # t-rows per phase-A chunk
CH_PER_B = T // CHUNK       # 4 chunks per sample
HB = H // 128               # 8 h blocks
JJ = CHUNK // 128           # 4 128-t slices per chunk
SPS = T // 128              # 16 slices per sample

_cache = {}


def build():
    import concourse.bass as bass
    import concourse.tile as tile
    from concourse import bacc, mybir
    from contextlib import ExitStack

    f32 = mybir.dt.float32
    bf16 = mybir.dt.bfloat16
    AF = mybir.ActivationFunctionType
    AX = mybir.AxisListType

    nc = bacc.Bacc("TRN2", target_bir_lowering=False, debug=False)

    x = nc.declare_dram_parameter("x", [BC, T, H], f32, isOutput=False)
    w = nc.declare_dram_parameter("w", [H, H], f32, isOutput=False)
    c = nc.declare_dram_parameter("c", [H, 1], f32, isOutput=False)
    out_ctx = nc.declare_dram_parameter("out_ctx", [BC, H], f32, isOutput=True)
    out_attn = nc.declare_dram_parameter("out_attn", [BC, T], f32, isOutput=True)

    xf = x[:].rearrange("b t h -> (b t) h")  # [8192, 1024]

    with tile.TileContext(nc) as tc, ExitStack() as ctx:
        ep = ctx.enter_context
        const_pool = ep(tc.tile_pool(name="const", bufs=1))
        xt_pool = ep(tc.tile_pool(name="xt", bufs=24))
        st_pool = ep(tc.tile_pool(name="st", bufs=16))
        junk_pool = ep(tc.tile_pool(name="junk", bufs=2))
        sm_pool = ep(tc.tile_pool(name="sm", bufs=2))
        dram_pool = ep(tc.tile_pool(name="dram", bufs=4, space="DRAM"))
        mm_ps = ep(tc.tile_pool(name="mm_ps", bufs=3, space="PSUM"))
        lg_ps = ep(tc.tile_pool(name="lg_ps", bufs=1, space="PSUM"))
        abc_ps = ep(tc.tile_pool(name="abc_ps", bufs=2, space="PSUM"))
        tiny_ps = ep(tc.tile_pool(name="tiny_ps", bufs=2, space="PSUM"))

        # ---- constants / weights ----
        w_sb = const_pool.tile([128, HB, H], bf16, tag="w")
        nc.gpsimd.dma_start(
            out=w_sb[:], in_=w[:].rearrange("(hb p) h -> p hb h", p=128)
        )
        c_sb = const_pool.tile([128, HB], bf16, tag="c")
        nc.gpsimd.dma_start(
            out=c_sb[:], in_=c[:].rearrange("(a p) k -> p (a k)", p=128)
        )
        ones_col = const_pool.tile([128, 1], f32, tag="ones_col")
        nc.any.memset(ones_col[:], 1.0)
        ones_row = const_pool.tile([1, 128], f32, tag="ones_row")
        nc.any.memset(ones_row[:], 1.0)
        ones_row_b = const_pool.tile([1, 128], bf16, tag="ones_row_b")
        nc.any.memset(ones_row_b[:], 1.0)
        ident_f32 = const_pool.tile([128, 128], f32, tag="ident")
        from concourse.masks import make_identity

        make_identity(nc, ident_f32[:])
        probs = const_pool.tile([128, BC * SPS], f32, tag="probs")

        xbf_by_b = [None] * BC
        cast_insts_by_b = [None] * BC

        def sample_tail(b, xts):
            """softmax + pooling for sample b (all its logits are in probs)."""
            pcols = probs[:, b * SPS : (b + 1) * SPS]
            partial = sm_pool.tile([128, 1], f32, tag="partial")
            nc.vector.reduce_sum(partial[:], pcols, axis=AX.X)
            tps = tiny_ps.tile([1, 1], f32, tag="tiny")
            nc.tensor.matmul(tps[:], ones_col[:], partial[:], start=True, stop=True)
            tsb = sm_pool.tile([1, 1], f32, tag="tsb")
            nc.vector.tensor_copy(tsb[:], tps[:])
            bps = tiny_ps.tile([128, 1], f32, tag="tiny")
            nc.tensor.matmul(bps[:], ones_row[:], tsb[:], start=True, stop=True)
            rsb = sm_pool.tile([128, 1], f32, tag="rsb")
            nc.vector.reciprocal(rsb[:], bps[:])
            attn_f = sm_pool.tile([128, SPS], f32, tag="attn_f")
            nc.vector.tensor_scalar_mul(attn_f[:], pcols, rsb[:])
            # transpose attn to [16 slices, 128 t] rows: contiguous output DMA
            atp = tiny_ps.tile([SPS, 128], f32, tag="tiny")
            nc.tensor.transpose(atp[:], attn_f[:], ident_f32[:])
            arow = sm_pool.tile([SPS, 128], f32, tag="arow")
            nc.vector.tensor_copy(arow[:], atp[:])
            nc.scalar.dma_start(
                out=out_attn[b : b + 1, :].rearrange("o (s p) -> (o s) p", p=128),
                in_=arow[:],
            )
            # bounce attn through DRAM scratch to refold [16,128] -> [1,2048]
            ascr = dram_pool.tile([1, T], f32, tag="ascr", name=f"ascr{b}")
            wi = nc.scalar.dma_start(
                out=ascr[:].rearrange("o (s p) -> (o s) p", p=128), in_=arow[:]
            )
            a16 = sm_pool.tile([1, T], bf16, tag="a16")
            ri = nc.gpsimd.dma_start(out=a16[:], in_=ascr[:])
            tile.add_dep_helper(ri.ins, wi.ins, reason="ascr RAW")
            # broadcast attn row across partitions: ab[p, t] = attn[t]
            ab = sm_pool.tile([128, T], bf16, tag="ab")
            for q in range(CH_PER_B):
                abp = abc_ps.tile([128, 512], f32, tag="abc")
                nc.tensor.matmul(
                    abp[:],
                    ones_row_b[:],
                    a16[0:1, q * 512 : (q + 1) * 512],
                    start=True,
                    stop=True,
                )
                nc.vector.tensor_copy(ab[:, q * 512 : (q + 1) * 512], abp[:])
            # pooling on DVE: ctxT[h, hb] = sum_t XT[hb][h, t] * attn[t]
            ctxT = sm_pool.tile([128, HB], f32, tag="ctxT")
            for hb in range(HB):
                junk = junk_pool.tile([128, T], bf16, tag="junk")
                nc.vector.tensor_mul(junk[:], xts[hb][:], ab[:])
                nc.vector.reduce_sum(ctxT[:, hb : hb + 1], junk[:], axis=AX.X)
            nc.scalar.dma_start(
                out=out_ctx[b : b + 1, :].rearrange("o (hb p) -> p (o hb)", p=128),
                in_=ctxT[:],
            )

        def main_mms(b, ch, xts):
            """main matmuls + tanh for chunk ch of sample b; returns st tiles."""
            sts = []
            for hob in range(HB):
                ps = mm_ps.tile([128, CHUNK], f32, tag="mm")
                for hib in range(HB):
                    nc.tensor.matmul(
                        ps[:],
                        w_sb[:, hib, hob * 128 : (hob + 1) * 128],
                        xts[hib][:, ch * CHUNK : (ch + 1) * CHUNK],
                        start=(hib == 0),
                        stop=(hib == HB - 1),
                    )
                st = st_pool.tile([128, CHUNK], bf16, tag="st")
                nc.scalar.activation(st[:], ps[:], AF.Tanh)
                sts.append(st)
            return sts

        def logits_part(b, ch, sts):
            """logits^T matmuls + exp for chunk ch of sample b."""
            lg = lg_ps.tile([128, JJ], f32, tag="lg")
            for j in range(JJ):
                for hob in range(HB):
                    nc.tensor.matmul(
                        lg[:, j : j + 1],
                        sts[hob][:, j * 128 : (j + 1) * 128],
                        c_sb[:, hob : hob + 1],
                        start=(hob == 0),
                        stop=(hob == HB - 1),
                    )
            gs = b * SPS + ch * JJ  # global slice index
            nc.scalar.activation(probs[:, gs : gs + JJ], lg[:], AF.Exp)

        def emit_casts(b):
            """P0: cast sample b's x f32 -> bf16 natural-layout DRAM scratch.
            Contiguous reads/writes — one cast per 512-row chunk."""
            xbf = dram_pool.tile([T, H], bf16, tag="xbf", name=f"xbf{b}")
            xbf_by_b[b] = xbf
            cast_insts = []
            for ch in range(CH_PER_B):
                ci = nc.gpsimd.dma_start(
                    out=xbf[ch * CHUNK : (ch + 1) * CHUNK, :],
                    in_=xf[b * T + ch * CHUNK : b * T + (ch + 1) * CHUNK, :],
                )
                cast_insts.append(ci)
            cast_insts_by_b[b] = cast_insts

        def emit_transposes(b, half, xts=None):
            # XT: xbar transposes, all on the SP ring — two concurrent xbar
            # streams (sync + scalar) corrupt data. The strided-row source
            # (256B rows, 2KB stride) is the xbar's supported mid-dim case.
            if xts is None:
                xts = [
                    xt_pool.tile([128, T], bf16, tag="xt", name=f"xt{b}_{hb}")
                    for hb in range(HB)
                ]
            t0, t1 = half * (T // 2), (half + 1) * (T // 2)
            for hb in range(HB):
                ti = nc.sync.dma_start(
                    out=xts[hb][:, t0:t1],
                    in_=xbf_by_b[b][t0:t1, hb * 128 : (hb + 1) * 128],
                    transpose=True,
                )
                for ci in cast_insts_by_b[b][half * 2 : half * 2 + 2]:
                    tile.add_dep_helper(ti.ins, ci.ins, reason="xnat RAW")
            return xts

        # Software-pipelined emission: casts one sample ahead; transposes a
        # half-sample ahead; logits one chunk behind main MMs; sample tails
        # lag into the next sample's second chunk.
        emit_casts(0)
        xts_by_b = {0: emit_transposes(0, 0)}
        emit_transposes(0, 1, xts_by_b[0])
        pend_lg = None   # (b, ch, sts) awaiting logits emission
        pend_tail = None  # sample awaiting softmax+pooling emission
        for b in range(BC):
            if b + 1 < BC:
                emit_casts(b + 1)
                xts_by_b[b + 1] = emit_transposes(b + 1, 0)
            for ch in range(CH_PER_B):
                if ch == 2 and b + 1 < BC:
                    emit_transposes(b + 1, 1, xts_by_b[b + 1])
                sts = main_mms(b, ch, xts_by_b[b])
                if pend_lg is not None:
                    logits_part(*pend_lg)
                pend_lg = (b, ch, sts)
                if ch == 1 and pend_tail is not None:
                    sample_tail(pend_tail, xts_by_b[pend_tail])
                    pend_tail = None
            xts_by_b.pop(b - 1, None)
            pend_tail = b
        logits_part(*pend_lg)
        sample_tail(pend_tail, xts_by_b[pend_tail])

    nc.compile()
    return nc


def _get_nc():
    if "nc" not in _cache:
        _cache["nc"] = build()
    return _cache["nc"]


def kernel(gru_output, attention_weights, context_vector):
    from concourse.bass_utils import run_bass_kernel_spmd

    nc = _get_nc()
    in_maps = []
    for i in range(NCORES):
        in_maps.append(
            {
                "x": np.ascontiguousarray(
                    gru_output[i * BC : (i + 1) * BC], dtype=np.float32
                ),
                "w": np.ascontiguousarray(attention_weights, dtype=np.float32),
                "c": np.ascontiguousarray(context_vector, dtype=np.float32),
            }
        )
    res = run_bass_kernel_spmd(nc, in_maps, list(range(NCORES))).results
    context = np.concatenate([res[i]["out_ctx"] for i in range(NCORES)], axis=0)
    attn = np.concatenate([res[i]["out_attn"] for i in range(NCORES)], axis=0)
    return context, attn
